# revision 2
# baseline (speedup 1.0000x reference)
"""Trainium2 Bass kernel for nn_Logic_Learning_Model (declarative logic-rule
point-process log-likelihood).

Algorithm (factorized, validated vs reference at ~4e-7 rel err in numpy):
For each sample, all features are masked weighted sums over per-predicate
event arrays evaluated at 512 query times (255 head-event times + 1 pad +
256 grid points):

  feat0(q) = e^{2(Ck-tq)} * sum_j [t1_j < tq-TOL] * g_j(Ck)
             g_j(Ck) = [s1_j==1] * e^{2(t1_j-Ck)} * What_j
             What_j  = e^{C2-t1_j} * sum_i [t0_i < t1_j-TOL][s0_i==1] e^{t0_i-C2}
  feat1(q) = e^{Ck-tq} * sum_j [t2_j < tq-TOL] * [s2_j==1] e^{t2_j-Ck}
  feat2(q) = e^{Ck-tq} * ( D'(q) - C'(q) ),  D' = sum [t3<=tq] v3,
             C' = sum [(tq-t3)>TOL] v3,  v3_j = [s3_j==0] e^{t3_j-Ck}
  sh[idx(q)] = sum_j [th_j < tq] * (sh_j - sh_{j-1,wrap}) + sh_255

Ck is a per-query-block shift (C1=38.4 for tq<38.4, C2=76.8 otherwise) to
keep every exponential inside fp32 range; both variants are computed and
selected per query.  Masks are exact 0/1 bf16 tiles built by fp32 compares
with the same rounding as the reference; weighted sums run on the PE as
bf16 matmuls with Dekker-split (hi+lo) weight vectors accumulating in fp32
PSUM.

Sharding: pure data parallel, 32 samples per core on 8 cores; each core
returns 128 per-(sample,query-tile) partial sums; host adds them up.
"""

import numpy as np

import concourse.bass as bass
import concourse.mybir as mybir
from concourse.tile import TileContext

F32 = mybir.dt.float32
BF16 = mybir.dt.bfloat16
I32 = mybir.dt.int32
U8 = mybir.dt.uint8

NCORES = 8
S = 32          # samples per core
E = 256         # events per predicate
EH = 128        # half (one partition tile)
Q = 512         # padded query count: 255 head + 1 pad + 256 grid
T_MAX = 76.8
RES = 0.3
TOL = 0.1
C1 = 38.4
C2 = 76.8

AX = mybir.AxisListType
OP = mybir.AluOpType
ACTF = mybir.ActivationFunctionType


def bcast(ap, n=128):
    """0-stride partition broadcast view of a flat DRAM AP."""
    return bass.AP(ap.tensor, ap.offset, [[0, n]] + list(ap.ap))


def build_nc():
    from concourse.bacc import Bacc
    nc = Bacc(None, target_bir_lowering=False)
    times_d = nc.dram_tensor("times", [S, 5, E], F32, kind="ExternalInput")
    states_d = nc.dram_tensor("states", [S, 5, E], I32, kind="ExternalInput")
    base_d = nc.dram_tensor("base", [1], F32, kind="ExternalInput")
    weights_d = nc.dram_tensor("weights", [3], F32, kind="ExternalInput")
    grid_d = nc.dram_tensor("grid", [E], F32, kind="ExternalInput")
    # grid rows pre-replicated for the post-phase query matrix (constant)
    gridq_d = nc.dram_tensor("gridq", [2, S, EH], F32, kind="ExternalInput")
    # consts[:, 0] = qtmask (1 for head rows), consts[:, 1] = pad column mask
    consts_d = nc.dram_tensor("consts", [128, 2], F32, kind="ExternalInput")
    out_d = nc.dram_tensor("out", [128], F32, kind="ExternalOutput")

    with TileContext(nc) as tc:
        _build(tc, nc, times_d, states_d, base_d, weights_d, grid_d, gridq_d,
               consts_d, out_d)
    nc.finalize()
    return nc


def _build(tc, nc, times_d, states_d, base_d, weights_d, grid_d, gridq_d,
           consts_d, out_d):
    cp = tc.alloc_tile_pool(name="const", bufs=1)
    sp = tc.alloc_tile_pool(name="samp", bufs=3)
    qp = tc.alloc_tile_pool(name="qbc", bufs=S)
    mp = tc.alloc_tile_pool(name="mask", bufs=3)
    pp = tc.alloc_tile_pool(name="psum", bufs=1, space="PSUM")
    pw = tc.alloc_tile_pool(name="psumw", bufs=2, space="PSUM")

    # ---------------- phase 0: load events + batched prep ----------------
    # per-(array, half) event/state tiles, [128 events, 32 samples]
    T = {}
    ST = {}
    for a in range(5):
        for kt in range(2):
            t_t = cp.tile([EH, S], F32, tag=f"T{a}{kt}", name="t")
            s_t = cp.tile([EH, S], I32, tag=f"S{a}{kt}", name="t")
            src_t = times_d[:, a, kt * EH:(kt + 1) * EH].rearrange("s p -> p s")
            src_s = states_d[:, a, kt * EH:(kt + 1) * EH].rearrange("s p -> p s")
            nc.sync.dma_start(out=t_t[:], in_=src_t)
            nc.sync.dma_start(out=s_t[:], in_=src_s)
            T[a, kt] = t_t
            ST[a, kt] = s_t

    ones_col = cp.tile([128, 1], F32, tag="ones", name="t")
    nc.vector.memset(ones_col[:], 1.0)

    # base/weights broadcast columns (0-stride DMA from DRAM)
    wbbc = cp.tile([128, 4], F32, tag="wbbc", name="t")
    nc.vector.memset(wbbc[:], 0.0)
    nc.sync.dma_start(out=wbbc[:, 0:3], in_=bcast(weights_d[:]))
    nc.sync.dma_start(out=wbbc[:, 3:4], in_=bcast(base_d[:]))
    negw2 = cp.tile([128, 1], F32, tag="negw2", name="t")
    nc.vector.tensor_scalar(out=negw2[:], in0=wbbc[:, 2:3], scalar1=-1.0,
                            scalar2=None, op0=OP.mult)

    # consts: col0 = query-type mask (1.0 head rows), col1 = pad-column mask
    consts = cp.tile([128, 2], F32, tag="consts", name="t")
    nc.sync.dma_start(out=consts[:], in_=consts_d[:])
    qtmask = consts[:, 0:1]
    padcol = consts[:, 1:2]

    # ---- batched exponentials / state masks / weight vectors per half ----
    ew = {}     # exp tiles keyed by (name, kt)
    sm = {}
    for kt in range(2):
        # exp args -> one tile per needed exponential, [128, 32]
        def _exp(tag, src, scale, off):
            arg = sp.tile([EH, S], F32, tag=f"arg{tag}{kt}", name="t")
            nc.vector.tensor_scalar(out=arg[:], in0=src[:], scalar1=scale,
                                    scalar2=off, op0=OP.mult, op1=OP.add)
            e_t = cp.tile([EH, S], F32, tag=f"e{tag}{kt}", name="t")
            nc.scalar.activation(e_t[:], arg[:], ACTF.Exp)
            return e_t

        ew["w0", kt] = _exp("w0", T[0, kt], 1.0, -C2)       # e^{t0-C2}
        ew["c2t1", kt] = _exp("c2t1", T[1, kt], -1.0, C2)   # e^{C2-t1}
        ew["g1", kt] = _exp("g1", T[1, kt], 2.0, -2.0 * C1)  # e^{2(t1-C1)}
        ew["g2", kt] = _exp("g2", T[1, kt], 2.0, -2.0 * C2)
        ew["v21", kt] = _exp("v21", T[2, kt], 1.0, -C1)
        ew["v22", kt] = _exp("v22", T[2, kt], 1.0, -C2)
        ew["v31", kt] = _exp("v31", T[3, kt], 1.0, -C1)
        ew["v32", kt] = _exp("v32", T[3, kt], 1.0, -C2)

        for a, val, tag in ((0, 1, "s0"), (1, 1, "s1"), (2, 1, "s2"), (3, 0, "s3")):
            m = cp.tile([EH, S], F32, tag=f"{tag}{kt}", name="t")
            nc.vector.tensor_scalar(out=m[:], in0=ST[a, kt][:], scalar1=val,
                                    scalar2=None, op0=OP.is_equal)
            sm[tag, kt] = m

        # [t3 <= C1]: zeroes v3C1 entries that no blk1 query can ever select;
        # keeps sum(v3C1) small so the D' sign fixup doesn't cancel.
        m31 = cp.tile([EH, S], F32, tag=f"m31{kt}", name="t")
        nc.vector.tensor_scalar(out=m31[:], in0=T[3, kt][:], scalar1=C1,
                                scalar2=None, op0=OP.is_le)
        sm["m31", kt] = m31

        # negated t3 (ACT sign bias) — for the D' masks
        nt3 = cp.tile([EH, S], F32, tag=f"nt3{kt}", name="t")
        nc.vector.tensor_scalar(out=nt3[:], in0=T[3, kt][:], scalar1=-1.0,
                                scalar2=None, op0=OP.mult)
        sm["nt3", kt] = nt3

    def dekker(dst, blk0, src32, tmp_tag):
        """write bf16 (hi, lo) blocks of src32 [128, S] into dst block cols
        [blk0*S:(blk0+1)*S] and [(blk0+1)*S:(blk0+2)*S]"""
        hi = dst[:, blk0 * S:(blk0 + 1) * S]
        lo = dst[:, (blk0 + 1) * S:(blk0 + 2) * S]
        nc.vector.tensor_copy(out=hi, in_=src32[:])
        tmp = sp.tile([EH, S], F32, tag=tmp_tag, name="t")
        nc.vector.tensor_copy(out=tmp[:], in_=hi)
        nc.vector.tensor_tensor(out=lo, in0=src32[:], in1=tmp[:],
                                op=OP.subtract)

    # w0 pairs (feat0 inner sum weights), [128, 2*S]: cols 2s,2s+1 = h,l
    w0pair = {}
    for kt in range(2):
        w0 = sp.tile([EH, S], F32, tag=f"w0m{kt}", name="t")
        nc.vector.tensor_tensor(out=w0[:], in0=ew["w0", kt][:], in1=sm["s0", kt][:],
                                op=OP.mult)
        pair = cp.tile([EH, 2 * S], BF16, tag=f"w0pair{kt}", name="t")
        dekker(pair, 0, w0, f"w0tmp{kt}")
        w0pair[kt] = pair

    # v2 / v3 quads [128, 4*S]: cols 4s..4s+3 = [vC1h vC1l vC2h vC2l]
    vB = {}
    vC = {}
    for kt in range(2):
        q_b = cp.tile([EH, 4 * S], BF16, tag=f"vB{kt}", name="t")
        q_c = cp.tile([EH, 4 * S], BF16, tag=f"vC{kt}", name="t")
        for ver, (e2tag, e3tag) in enumerate((("v21", "v31"), ("v22", "v32"))):
            v2 = sp.tile([EH, S], F32, tag=f"v2m{kt}{ver}", name="t")
            nc.vector.tensor_tensor(out=v2[:], in0=ew[e2tag, kt][:],
                                    in1=sm["s2", kt][:], op=OP.mult)
            dekker(q_b, 2 * ver, v2, f"dkb{kt}{ver}")
            v3 = sp.tile([EH, S], F32, tag=f"v3m{kt}{ver}", name="t")
            nc.vector.tensor_tensor(out=v3[:], in0=ew[e3tag, kt][:],
                                    in1=sm["s3", kt][:], op=OP.mult)
            if ver == 0:
                nc.vector.tensor_tensor(out=v3[:], in0=v3[:],
                                        in1=sm["m31", kt][:], op=OP.mult)
            dekker(q_c, 2 * ver, v3, f"dkc{kt}{ver}")
        vB[kt] = q_b
        vC[kt] = q_c

    # dsh (bf16): sh_j - sh_{j-1 (wrap)}; stored zero-padded [z z z z dsh] per
    # sample so the E matmul can share the D' 32-partition psum group.
    shm1 = {0: sp.tile([EH, S], I32, tag="shm10", name="t"), 1: sp.tile([EH, S], I32, tag="shm11", name="t")}
    nc.vector.memset(shm1[0][:], 0)
    nc.vector.memset(shm1[1][:], 0)
    nc.sync.dma_start(out=shm1[0][1:128, :], in_=ST[4, 0][0:127, :])
    nc.sync.dma_start(out=shm1[0][0:1, :], in_=ST[4, 1][127:128, :])
    nc.sync.dma_start(out=shm1[1][1:128, :], in_=ST[4, 1][0:127, :])
    nc.sync.dma_start(out=shm1[1][0:1, :], in_=ST[4, 0][127:128, :])
    dsh = {}
    for kt in range(2):
        d = cp.tile([EH, 5 * S], BF16, tag=f"dsh{kt}", name="t")
        nc.vector.memset(d[:], 0.0)
        nc.vector.tensor_tensor(out=d[:, 4 * S:5 * S], in0=ST[4, kt][:],
                                in1=shm1[kt][:], op=OP.subtract)
        dsh[kt] = d

    # escol = 1 - 2*sh[255], per (sample,qt) partition column
    sh255row = sp.tile([1, S], I32, tag="sh255row", name="t")
    nc.sync.dma_start(out=sh255row[:], in_=ST[4, 1][127:128, :])
    esrow = cp.tile([1, S], F32, tag="esrow", name="t")
    nc.vector.tensor_scalar(out=esrow[:], in0=sh255row[:], scalar1=-2.0,
                            scalar2=1.0, op0=OP.mult, op1=OP.add)
    escol = cp.tile([128, 1], F32, tag="escol", name="t")
    nc.vector.memset(escol[:], 0.0)
    for qt in range(4):
        nc.sync.dma_start(out=escol[32 * qt:32 * (qt + 1), :], in_=esrow[0:1, :])

    # ------------- phase 1: per-sample What (feat0 inner sums) -------------
    wst = cp.tile([128, 4 * S], F32, tag="wst", name="t")  # cols 4s.. = [j0h j0l j1h j1l]
    tqbc_tiles = []
    for s in range(S):
        # query broadcast [128, Q]: [head th[1:256] | pad=th[255] | grid]
        tqbc = qp.tile([128, Q], F32, tag="tqbc", name="t")
        nc.vector.memset(tqbc[:], 0.0)
        nc.sync.dma_start(out=tqbc[:, 0:255], in_=bcast(times_d[s, 4, 1:256]))
        nc.sync.dma_start(out=tqbc[:, 255:256], in_=bcast(times_d[s, 4, 255:256]))
        nc.sync.dma_start(out=tqbc[:, 256:Q], in_=bcast(grid_d[:]))
        tqbc_tiles.append(tqbc)

        t1bc = sp.tile([128, E], F32, tag="t1bc", name="t")
        nc.sync.dma_start(out=t1bc[:], in_=bcast(times_d[s, 1, :]))

        psw = pw.tile([128, 4], F32, tag="psw", name="t")
        mwts = []
        for ikt in range(2):
            mwt = sp.tile([128, E], BF16, tag=f"mwt{ikt}", name="t")
            nc.vector.tensor_scalar(out=mwt[:], in0=t1bc[:], scalar1=TOL,
                                    scalar2=T[0, ikt][:, s:s + 1],
                                    op0=OP.subtract, op1=OP.is_gt)
            mwts.append(mwt)
        for jkt in range(2):
            for ikt in range(2):
                nc.tensor.matmul(psw[:, 2 * jkt:2 * jkt + 2],
                                 mwts[ikt][:, jkt * EH:(jkt + 1) * EH],
                                 w0pair[ikt][:, s::S][:, 0:2],
                                 start=(ikt == 0), stop=(ikt == 1))
        nc.vector.tensor_copy(out=wst[:, s::S][:, 0:4], in_=psw[:])

    # ------------- phase 2: batched g-vector assembly (feat0 weights) ------
    gA = {}
    for kt in range(2):
        wh = sp.tile([EH, S], F32, tag=f"wh{kt}", name="t")
        # wst blocks: [j0h | j0l | j1h | j1l], each S wide
        nc.vector.tensor_tensor(out=wh[:], in0=wst[:, 2 * kt * S:(2 * kt + 1) * S],
                                in1=wst[:, (2 * kt + 1) * S:(2 * kt + 2) * S],
                                op=OP.add)
        nc.vector.tensor_tensor(out=wh[:], in0=wh[:], in1=ew["c2t1", kt][:],
                                op=OP.mult)
        g_t = cp.tile([EH, 4 * S], BF16, tag=f"gA{kt}", name="t")
        for ver, etag in enumerate(("g1", "g2")):
            g32 = sp.tile([EH, S], F32, tag=f"g32{kt}{ver}", name="t")
            nc.vector.tensor_tensor(out=g32[:], in0=ew[etag, kt][:], in1=wh[:],
                                    op=OP.mult)
            nc.vector.tensor_tensor(out=g32[:], in0=g32[:], in1=sm["s1", kt][:],
                                    op=OP.mult)
            dekker(g_t, 2 * ver, g32, f"dkg{kt}{ver}")
        gA[kt] = g_t

    # ------------- phase 3: per-sample masks + weighted sums ---------------
    # psum groups (32-partition aligned): A@0-3, B@32-35, C'@64-67 in bank 0;
    # D'@0-3 + E@4 in bank 1 (E first via the zero-padded dsh lhsT).
    # Two persistent psum tiles (memset once so whole-range reads are defined).
    stage2 = cp.tile([128, 20 * 128], F32, tag="stage2", name="t")
    nc.vector.memset(stage2[:], 0.0)
    psums = []
    for i in range(2):
        t_ps = pp.tile([128, 2 * Q], F32, tag=f"pm{i}", name="t")
        nc.vector.memset(t_ps[:], 0.0)
        psums.append(t_ps)
    for s in range(S):
        tqbc = tqbc_tiles[s]
        tqp = sp.tile([128, Q], F32, tag="tqp", name="t")  # fl(tq - 0.1)
        nc.vector.tensor_scalar(out=tqp[:], in0=tqbc[:], scalar1=TOL,
                                scalar2=None, op0=OP.subtract)
        psum = psums[s % 2]
        for kt in range(2):
            mA = mp.tile([128, Q], BF16, tag=f"mA{kt}", name="t")
            nc.vector.tensor_scalar(out=mA[:], in0=tqp[:],
                                    scalar1=T[1, kt][:, s:s + 1], scalar2=None,
                                    op0=OP.is_gt)
            mB = mp.tile([128, Q], BF16, tag=f"mB{kt}", name="t")
            nc.vector.tensor_scalar(out=mB[:], in0=tqp[:],
                                    scalar1=T[2, kt][:, s:s + 1], scalar2=None,
                                    op0=OP.is_gt)
            mC = mp.tile([128, Q], BF16, tag=f"mC{kt}", name="t")
            nc.vector.tensor_scalar(out=mC[:], in0=tqbc[:],
                                    scalar1=T[3, kt][:, s:s + 1], scalar2=TOL,
                                    op0=OP.subtract, op1=OP.is_gt)
            # D' 0/1 mask on gpsimd: [t3 <= tq]
            mD = mp.tile([128, Q], BF16, tag=f"mD{kt}", name="t")
            nc.gpsimd.tensor_scalar(out=mD[:], in0=tqbc[:],
                                    scalar1=T[3, kt][:, s:s + 1], scalar2=None,
                                    op0=OP.is_ge)
            # E on gpsimd: [tq > th]
            mE = mp.tile([128, Q], BF16, tag=f"mE{kt}", name="t")
            nc.gpsimd.tensor_scalar(out=mE[:], in0=tqbc[:],
                                    scalar1=T[4, kt][:, s:s + 1], scalar2=None,
                                    op0=OP.is_gt)
            st = (kt == 0)
            sp_ = (kt == 1)
            nc.tensor.matmul(psum[0:4, 0:Q], gA[kt][:, s::S][:, 0:4], mA[:],
                             start=st, stop=sp_)
            nc.tensor.matmul(psum[32:36, 0:Q], vB[kt][:, s::S][:, 0:4], mB[:],
                             start=st, stop=sp_)
            nc.tensor.matmul(psum[64:68, 0:Q], vC[kt][:, s::S][:, 0:4], mC[:],
                             start=st, stop=sp_)
            # D'+E share bank-1 rows 0-4 (E via the zero-padded dsh lhsT).
            # E opens (kt0, rows 0-4) and closes (kt1, stop) the group so the
            # whole row range is covered by start/stop.
            nc.tensor.matmul(psum[0:5, Q:2 * Q], dsh[kt][:, s::S][:, 0:5],
                             mE[:], start=st, stop=sp_)
            nc.tensor.matmul(psum[0:4, Q:2 * Q], vC[kt][:, s::S][:, 0:4],
                             mD[:], start=False, stop=False,
                             skip_group_check=True)
        stga = sp.tile([128, Q], F32, tag="stga", name="t")
        nc.scalar.copy(stga[0:69, :], psum[0:69, 0:Q])
        stgb = sp.tile([5, Q], F32, tag="stgb", name="t")
        nc.vector.tensor_copy(out=stgb[:], in_=psum[0:5, Q:2 * Q])
        for qt in range(4):
            row = 32 * qt + s
            dst = stage2[row:row + 1, :].rearrange(
                "one (r q) -> one r q", r=20)
            for g in range(3):
                nc.sync.dma_start(
                    out=dst[:, 5 * g:5 * g + 5, :],
                    in_=stga[32 * g:32 * g + 5, qt * 128:(qt + 1) * 128])
            nc.sync.dma_start(out=dst[:, 15:20, :],
                              in_=stgb[:, qt * 128:(qt + 1) * 128])

    # ------------- phase 4: batched post-processing ------------------------
    def R(r):
        return stage2[:, r * 128:(r + 1) * 128]

    # query matrix [128 (s,qt), 128]
    tq_m = cp.tile([128, 128], F32, tag="tqm", name="t")
    nc.vector.memset(tq_m[:], 0.0)
    nc.sync.dma_start(out=tq_m[0:32, :], in_=times_d[:, 4, 1:129])
    nc.sync.dma_start(out=tq_m[32:64, 0:127], in_=times_d[:, 4, 129:256])
    nc.sync.dma_start(out=tq_m[32:64, 127:128], in_=times_d[:, 4, 255:256])
    nc.sync.dma_start(out=tq_m[64:96, :], in_=gridq_d[0])
    nc.sync.dma_start(out=tq_m[96:128, :], in_=gridq_d[1])

    def tmp(tag):
        return cp.tile([128, 128], F32, tag=tag, name="t")

    # pairwise hi+lo sums (in place into the hi slot)
    # roles: 0-3 A quads, 5-8 B, 10-13 C', 15-18 D', 19 E (4, 9, 14 junk)
    for r in (0, 2, 5, 7, 10, 12, 15, 17):
        nc.vector.tensor_tensor(out=R(r), in0=R(r), in1=R(r + 1), op=OP.add)
    A1, A2, B1, B2, Cs1, Cs2, Dr1, Dr2 = (R(r) for r in (0, 2, 5, 7, 10, 12, 15, 17))

    blk = cp.tile([128, 128], U8, tag="blk", name="t")
    nc.vector.tensor_scalar(out=blk[:], in0=tq_m[:], scalar1=C1, scalar2=None,
                            op0=OP.is_ge)
    biasC1 = cp.tile([128, 1], F32, tag="biasC1", name="t")
    nc.vector.memset(biasC1[:], C1)
    biasC2 = cp.tile([128, 1], F32, tag="biasC2", name="t")
    nc.vector.memset(biasC2[:], C2)
    e1 = tmp("e1")
    nc.scalar.activation(e1[:], tq_m[:], ACTF.Exp, bias=biasC1[:], scale=-1.0)
    e2 = tmp("e2")
    nc.scalar.activation(e2[:], tq_m[:], ACTF.Exp, bias=biasC2[:], scale=-1.0)

    def sel(tag, on_true, on_false):
        o = tmp(tag)
        nc.vector.select(o, blk[:], on_true, on_false)
        return o

    esel = sel("esel", e2[:], e1[:])
    Asel = sel("Asel", A2, A1)
    Bsel = sel("Bsel", B2, B1)
    Csel = sel("Csel", Cs2, Cs1)
    Dsel = sel("Dsel", Dr2, Dr1)

    feat0 = tmp("feat0")
    nc.vector.tensor_tensor(out=feat0[:], in0=esel[:], in1=Asel[:], op=OP.mult)
    nc.vector.tensor_tensor(out=feat0[:], in0=feat0[:], in1=esel[:], op=OP.mult)
    feat1 = tmp("feat1")
    nc.vector.tensor_tensor(out=feat1[:], in0=esel[:], in1=Bsel[:], op=OP.mult)
    feat2 = tmp("feat2")
    nc.vector.tensor_tensor(out=feat2[:], in0=Dsel[:], in1=Csel[:], op=OP.subtract)
    nc.vector.tensor_tensor(out=feat2[:], in0=feat2[:], in1=esel[:], op=OP.mult)

    eff0 = tmp("eff0")
    nc.vector.tensor_scalar(out=eff0[:], in0=R(19), scalar1=-2.0, scalar2=escol[:],
                            op0=OP.mult, op1=OP.add)

    combo = tmp("combo")
    nc.vector.tensor_scalar(out=combo[:], in0=feat0[:], scalar1=wbbc[:, 0:1],
                            scalar2=None, op0=OP.mult)
    nc.vector.scalar_tensor_tensor(out=combo[:], in0=feat1[:], scalar=wbbc[:, 1:2],
                                   in1=combo[:], op0=OP.mult, op1=OP.add)
    nc.vector.scalar_tensor_tensor(out=combo[:], in0=feat2[:], scalar=negw2[:],
                                   in1=combo[:], op0=OP.mult, op1=OP.add)
    logits = tmp("logits")
    nc.vector.tensor_tensor(out=logits[:], in0=combo[:], in1=eff0[:], op=OP.mult)
    nc.vector.tensor_scalar(out=logits[:], in0=logits[:], scalar1=wbbc[:, 3:4],
                            scalar2=None, op0=OP.add)
    # zero the pad query (qt==1 rows, col 127) via the pad-column mask
    nc.vector.tensor_tensor(out=logits[:, 127:128], in0=logits[:, 127:128],
                            in1=padcol, op=OP.mult)

    hsum = cp.tile([128, 1], F32, tag="hsum", name="t")
    nc.vector.tensor_reduce(out=hsum[:], in_=logits[:], axis=AX.X, op=OP.add)
    expt = tmp("expt")
    intcol = cp.tile([128, 1], F32, tag="intcol", name="t")
    nc.scalar.activation(expt[:], logits[:], ACTF.Exp, accum_out=intcol[:])
    nc.vector.tensor_scalar(out=intcol[:], in0=intcol[:], scalar1=-RES,
                            scalar2=None, op0=OP.mult)
    qtmaski = cp.tile([128, 1], U8, tag="qtmaski", name="t")
    nc.vector.tensor_scalar(out=qtmaski[:], in0=qtmask, scalar1=0.5,
                            scalar2=None, op0=OP.is_ge)
    rowpart = cp.tile([128, 1], F32, tag="rowpart", name="t")
    nc.vector.select(rowpart[:], qtmaski[:], hsum[:], intcol[:])
    nc.sync.dma_start(out=out_d[:], in_=rowpart[:])

    for pool in (pw, pp, mp, qp, sp, cp):
        pool.release()


_NC_CACHE = []


def _get_nc():
    if not _NC_CACHE:
        _NC_CACHE.append(build_nc())
    return _NC_CACHE[0]


def make_inputs_for_core(times, states, base, weights, core):
    grid = np.arange(0.0, T_MAX, RES, dtype=np.float32)
    gridq = np.stack([np.tile(grid[0:128], (S, 1)), np.tile(grid[128:256], (S, 1))])
    consts = np.ones((128, 2), np.float32)
    consts[64:128, 0] = 0.0   # qtmask: 0 for grid rows (qt 2,3 blocks)
    consts[32:64, 1] = 0.0    # pad-column mask: 0 for qt1 block
    sl = slice(core * S, (core + 1) * S)
    return {
        "times": np.ascontiguousarray(times[sl]).astype(np.float32),
        "states": np.ascontiguousarray(states[sl]).astype(np.int32),
        "base": np.asarray(base, np.float32),
        "weights": np.asarray(weights, np.float32),
        "grid": grid,
        "gridq": np.ascontiguousarray(gridq).astype(np.float32),
        "consts": consts,
    }


def kernel(times, states, base, weights):
    from concourse.bass_utils import run_bass_kernel_spmd

    times = np.asarray(times, np.float32)
    states = np.asarray(states, np.int32)
    nc = _get_nc()
    in_maps = [make_inputs_for_core(times, states, base, weights, c)
               for c in range(NCORES)]
    res = run_bass_kernel_spmd(nc, in_maps, list(range(NCORES)))
    parts = np.stack([np.asarray(res.results[c]["out"]) for c in range(NCORES)])
    total = np.sum(parts.astype(np.float32), dtype=np.float32)
    return np.array([total], dtype=np.float32)


def run_traced(times, states, base, weights):
    """Profiled run; returns HW exec time in ns (or None if tracing off)."""
    from concourse.bass_utils import run_bass_kernel_spmd

    times = np.asarray(times, np.float32)
    states = np.asarray(states, np.int32)
    nc = _get_nc()
    in_maps = [make_inputs_for_core(times, states, base, weights, c)
               for c in range(NCORES)]
    res = run_bass_kernel_spmd(nc, in_maps, list(range(NCORES)), trace=True)
    return res

